# revision 1
# baseline (speedup 1.0000x reference)
"""Trainium2 Bass kernel for AdjacencyErrorAwareLoss.

Math (reference):
    A_fid = (d_hw == 1) * max(1 - d_error, 0)                    [128,128]
    scores[b,e] = P[b,i_e,:] @ A_fid @ P[b,j_e,:]                [B,E]
    loss = -mean_b( sum_e(w*scores) / max(sum_e w, 1e-8) )

Key transformation: scores[b,e] = S_b[i_e, j_e] where S_b = P_b @ A @ P_b^T.
Per sample: two 128^3 matmuls build S_b, then a weighted gather of E=4096
scalars from the 128x128 score matrix.

Distribution: data-parallel over B=64: 8 NeuronCores x 8 samples. On each
core, sample c is handled by GPSIMD core c (partitions 16c..16c+16).

Gather strategy (ap_gather: 8 GPSIMD cores, each processing its 16
partitions with a shared per-core index list, wrapped (s p) across the
core's partitions; the ucode walks all 16 partitions serially, ~40us per
4096 indices, which dominates the kernel):
  - partition p = 16c+q holds a masked 16384-entry table:
    table2[p, i*128+j] = S_c[i, j] if i//8 == q else 0, so a single
    shared index idx = i*128+j returns the right value on exactly one
    partition of the group and zero on the other 15 -- no separate
    selector-mask gather or mask multiply is needed.
  - the table is built by zeroing once (invariant background), then 16
    per-q stripe DMAs from a DRAM bounce of the S matrices (SBUF DMAs
    cannot collapse partitions or vary free offsets per partition).
  - summing each 16-partition group on the PE (lhsT = block-ones,
    rhs = 512-column slices of the gathered values) yields per-edge
    scores y[c, v]; psum column v of slice u is edge
    256*(v%16) + 32*u + v//16, so the weighted sum is one
    scalar_tensor_tensor per slice with a strided view of the natural
    w layout and a fused accumulator.
  - gathers are chunked x4 so the PE/DVE reduction of chunk i overlaps
    the GPSIMD gather of chunk i+1.
"""

import numpy as np

B, NL, NP, E = 64, 128, 128, 4096
N_CORES = 8
BPC = B // N_CORES  # samples per NeuronCore


def _patch_tile_drain():
    """This toolchain's walrus rejects >1 sem wait on a Drain; split the
    kernel-tail drain into one drain per pending semaphore."""
    import concourse.tile as tile
    from concourse.vector_clock import ScopedClock, VectorClock

    def _drain_and_barrier_split(self, tick_clock, wait_clock):
        nc = self.nc
        gc = tick_clock.global_clock  # VectorClock
        n = len(gc)
        for p in [i for i in range(n) if gc[i] > 0]:
            vec = VectorClock([gc[i] if i == p else 0 for i in range(n)])
            drain_inst = nc.sync.drain()
            wait_clock.add_sem_waits(drain_inst.ins, ScopedClock({None: vec}))
        nc.all_engine_barrier()
        assert self.sems is not None
        popped = nc._tile_sem_poison_stack.pop()
        assert popped is self._sem_poison
        nc.clear_and_free_semaphores(list(self.sems.allocated().values()))
        nc.all_engine_barrier()

    tile.TileContext._drain_and_barrier = _drain_and_barrier_split


def _split_multi_waits(nc, mybir):
    """Walrus codegen accepts at most one sem wait per instruction ("Too
    many sync wait commands"). Hoist extra waits onto preceding same-engine
    NoOps (engines execute in order, so this blocks equivalently)."""
    k = 0
    for f in nc.m.functions:
        for bb in f.blocks:
            insts = list(bb.instructions)
            out = []
            changed = False
            for ins in insts:
                si = ins.sync_info
                waits = list(si.on_wait) if si is not None and si.on_wait else []
                if len(waits) > 1:
                    changed = True
                    for w in waits[:-1]:
                        nop = mybir.InstNoOp(name=f"xw-{k}", ins=[], outs=[])
                        k += 1
                        nop.engine = ins.engine
                        nop.sync_info = mybir.SyncInfo(on_wait=[w], on_update=[])
                        nc.register_instruction(nop)
                        out.append(nop)
                    ins.sync_info = mybir.SyncInfo(
                        on_wait=[waits[-1]], on_update=list(si.on_update or [])
                    )
                out.append(ins)
            if changed:
                bb.instructions = out


def build_nc(repeat: int = 1, stage: str = "full"):
    """Build the Bass module (single-core SPMD program, run on 8 cores).

    repeat > 1 wraps the body in a hardware loop for timing runs.
    stage in ("loads", "mm", "gather", "full") truncates the body for
    cost bisection.
    """
    import concourse.bass as bass
    import concourse.mybir as mybir
    import concourse.tile as tile
    from concourse import library_config

    _patch_tile_drain()

    AL = mybir.AluOpType
    f32 = mybir.dt.float32
    i32 = mybir.dt.int32
    i16 = mybir.dt.int16

    nc = bass.Bass(detect_race_conditions=False)

    p_d = nc.dram_tensor("p", [BPC, NL, NP], f32, kind="ExternalInput")
    ep_d = nc.dram_tensor("ep", [BPC, E, 4], i32, kind="ExternalInput")
    w_d = nc.dram_tensor("w", [BPC, E], f32, kind="ExternalInput")
    derr_d = nc.dram_tensor("derr", [NP, NP], f32, kind="ExternalInput")
    dhw_d = nc.dram_tensor("dhw", [NP, NP], i32, kind="ExternalInput")
    out_d = nc.dram_tensor("out", [1, 1], f32, kind="ExternalOutput")

    # NEFF-embedded constants
    blockones_np = np.zeros((128, BPC), dtype=np.float32)
    for c in range(BPC):
        blockones_np[16 * c:16 * (c + 1), c] = 1.0
    blockones_d = nc.inline_tensor(blockones_np, name="blockones")
    ones_d = nc.inline_tensor(np.ones((128, 1), dtype=np.float32), name="ones128")
    ident_d = nc.inline_tensor(np.eye(128, dtype=np.float32), name="ident128")


    # gather chunk boundaries in 512-column blocks: tapering chunks so the
    # last gather (and the reduction tail it exposes) is small
    CHUNKS = [(0, 2), (2, 4), (4, 6), (6, 7), (7, 8)]

    with tile.TileContext(nc) as tc:
        with (
            tc.tile_pool(name="persist", bufs=1) as persist,
            tc.tile_pool(name="pp", bufs=2, space="PSUM") as pp,
            tc.tile_pool(name="vall", bufs=1, space="PSUM") as vallp,
            tc.tile_pool(name="pred", bufs=2, space="PSUM") as pred,
            tc.tile_pool(name="sdram", bufs=1, space="DRAM") as sdram,
        ):
            nc.gpsimd.load_library(library_config.ap_gather)

            # ---- persistent tiles
            blockones = persist.tile([128, BPC], f32)
            ones128 = persist.tile([128, 1], f32)
            ident = persist.tile([128, 128], f32)
            nc.sync.dma_start(blockones[:], blockones_d[:])
            nc.sync.dma_start(ones128[:], ones_d[:])
            nc.sync.dma_start(ident[:], ident_d[:])

            derr = persist.tile([128, 128], f32)
            dhw = persist.tile([128, 128], i32)
            nc.sync.dma_start(derr[:], derr_d[:])
            nc.sync.dma_start(dhw[:], dhw_d[:])

            pall = persist.tile([128, BPC, 128], f32)    # P, partition = row l
            pt_all = persist.tile([128, BPC, 128], f32)  # P^T per sample
            v_sb = persist.tile([128, BPC, 128], f32)    # V = (P A)^T per sample
            s_all = persist.tile([128, BPC, 128], f32)   # S copies (psum->sbuf)
            # masked gather table: [p, i*128+j] = S[i,j] if i//8 == p%16 else 0
            table2 = persist.tile([128, 16384], f32)
            epi = persist.tile([128, 256, 4], i32)       # edge pairs, contiguous
            idx16 = persist.tile([128, 256], i16)
            t1 = persist.tile([128, 256], i32)
            w_nat = persist.tile([BPC, E], f32)          # w natural layout
            # one tile per gather chunk so chunk i+1's gather write cannot
            # false-serialize against chunk i's reduction reads
            val2s = [persist.tile([128, (b - a) * 512, ], f32, name=f"val2_{i}")
                     for i, (a, b) in enumerate(CHUNKS)]
            afid = persist.tile([128, 128], f32)
            relu_e = persist.tile([128, 128], f32)
            mask_e = persist.tile([128, 128], f32)
            scr2 = persist.tile([BPC, 512], f32)         # stt elementwise out
            zaccs = persist.tile([BPC, 8], f32)          # per-chunk weighted sums
            accs = persist.tile([BPC, 16], f32) # chained ttr accums
            ws8 = persist.tile([BPC, 1], f32)
            zdiv = persist.tile([BPC, 1], f32)
            res = persist.tile([1, 1], f32)

            # zero the masked table once (stripes are rewritten in place every
            # iteration; the zero background is invariant)
            nc.vector.memset(table2[:, 0:8192], 0.0)
            nc.scalar.memzero(table2[:, 8192:16384])

            def body(_it=0):
                # ---- P first: it feeds the transpose->mm1->mm2 PE chain,
                # which is the longest pre-gather dependency path
                p_src = bass.AP(
                    tensor=p_d, offset=0,
                    ap=[[128, 128], [NL * NP, BPC], [1, 128]],
                )
                nc.sync.dma_start(pall[:], p_src)

                # ---- A_fid = (dhw == 1) * relu(1 - derr)
                nc.scalar.activation(
                    relu_e[:], derr[:],
                    mybir.ActivationFunctionType.Relu, bias=1.0, scale=-1.0,
                )
                nc.vector.tensor_scalar(
                    out=mask_e[:], in0=dhw[:], scalar1=1, scalar2=None,
                    op0=AL.is_equal,
                )
                nc.vector.tensor_tensor(
                    out=afid[:], in0=relu_e[:], in1=mask_e[:], op=AL.mult,
                )

                # ---- edge pairs, contiguous: partition 16c+r holds edges
                # [256r, 256r+256) of sample c; gather position k of sample c
                # is edge 256*(k%16) + k//16
                ep_src = bass.AP(
                    tensor=ep_d, offset=0,
                    ap=[[1024, 128], [4, 256], [1, 4]],
                )
                nc.sync.dma_start(epi[:], ep_src)
                # idx = i*128 + j  (i = int32 word 0, j = word 2)
                nc.vector.scalar_tensor_tensor(
                    out=t1[:], in0=epi[:, :, 0], scalar=128,
                    in1=epi[:, :, 2], op0=AL.mult, op1=AL.add,
                )
                nc.vector.tensor_copy(idx16[:], t1[:])

                # ---- w natural layout; wsum computed early (off critical path)
                nc.scalar.dma_start(w_nat[:], w_d[:])
                nc.vector.tensor_reduce(
                    out=ws8[:], in_=w_nat[:], axis=mybir.AxisListType.X, op=AL.add,
                )
                nc.vector.tensor_scalar(
                    out=ws8[:], in0=ws8[:], scalar1=1e-8, scalar2=None, op0=AL.max,
                )
                nc.vector.reciprocal(ws8[:], ws8[:])

                if stage == "loads":
                    nc.vector.memset(res[:], 0.0)
                    nc.sync.dma_start(out_d[:], res[:])
                    return

                # ---- per-sample transposes, then batched V = mm(A, P^T)
                for c in range(BPC):
                    pt_ps = pp.tile([128, 128], f32, tag="ptps")
                    nc.tensor.transpose(pt_ps[:], pall[:, c, :], ident[:])
                    nc.scalar.copy(pt_all[:, c, :], pt_ps[:])

                v_ps = vallp.tile([128, BPC, 128], f32)
                nc.tensor.matmul(
                    v_ps[:].rearrange("p a b -> p (a b)")[:, 0:512],
                    lhsT=afid[:],
                    rhs=pt_all[:].rearrange("p a b -> p (a b)")[:, 0:512],
                    start=True, stop=True,
                )
                nc.tensor.matmul(
                    v_ps[:].rearrange("p a b -> p (a b)")[:, 512:1024],
                    lhsT=afid[:],
                    rhs=pt_all[:].rearrange("p a b -> p (a b)")[:, 512:1024],
                    start=True, stop=True,
                )
                nc.vector.tensor_copy(v_sb[:], v_ps[:])

                # ---- S_c = V_c^T(as lhsT) @ P_c^T ; copy to s_all; bounce
                # (per-sample DRAM writes overlap the remaining matmuls)
                s_dr = sdram.tile([128, BPC, 128], f32, tag="sdram")
                for c in range(BPC):
                    s_ps = pp.tile([128, 128], f32, tag="sps")
                    nc.tensor.matmul(
                        s_ps[:], lhsT=v_sb[:, c, :], rhs=pt_all[:, c, :],
                        start=True, stop=True,
                    )
                    nc.scalar.copy(s_all[:, c, :], s_ps[:])
                    eng = nc.sync if c % 2 == 0 else nc.scalar
                    eng.dma_start(s_dr[:, c, :], s_all[:, c, :])
                # 16 stripe reads: table2[{16c+q}, 1024q : 1024q+1024] =
                #   S_c rows [8q, 8q+8)
                for q in range(16):
                    dst = table2[q::16, 1024 * q:1024 * (q + 1)]
                    eng = (nc.sync, nc.scalar, nc.gpsimd)[q % 3]
                    eng.dma_start(
                        dst,
                        s_dr[:].rearrange("l c x -> c l x")[:, 8 * q:8 * q + 8, :],
                    )

                if stage == "mm":
                    nc.vector.memset(res[:], 0.0)
                    nc.sync.dma_start(out_d[:], res[:])
                    return

                # ---- gather + reduction, chunked: the PE/DVE reduction of
                # chunk ch overlaps the GPSIMD gather of chunk ch+1
                for ch, (a, b) in enumerate(CHUNKS):
                    ss = slice(32 * a, 32 * b)
                    val2 = val2s[ch]
                    nc.gpsimd.ap_gather(
                        out_ap=val2[:].unsqueeze(2),
                        in_ap=table2[:].unsqueeze(2),
                        idxs_ap=idx16[:, ss],
                        channels=128, num_elems=16384, d=1,
                        num_idxs=(b - a) * 512,
                    )
                    if stage == "gonly":
                        continue
                    for u in range(a, b):
                        base = 512 * u
                        y_ps = pred.tile([BPC, 512], f32, tag="y")
                        nc.tensor.matmul(
                            y_ps[:], lhsT=blockones[:],
                            rhs=val2[:, base - 512 * a:base + 512 - 512 * a],
                            start=True, stop=True,
                        )
                        # w element for psum column v of this chunk:
                        #   w[cc, 256*(v%16) + 32*u + v//16]
                        win = w_nat[:].rearrange(
                            "c (lo s) -> c lo s", lo=16
                        )[:, :, 32 * u:32 * (u + 1)].rearrange("c lo v -> c v lo")
                        nc.vector.scalar_tensor_tensor(
                            out=scr2[:, 0:512].rearrange(
                                "c (v lo) -> c v lo", lo=16),
                            in0=y_ps[:].rearrange("c (v lo) -> c v lo", lo=16),
                            scalar=0.0, in1=win,
                            op0=AL.add, op1=AL.mult,
                            accum_out=zaccs[:, u:u + 1],
                        )

                if stage == "gonly":
                    nc.vector.tensor_copy(res[:], val2s[0][0:1, 0:1])
                    nc.sync.dma_start(out_d[:], res[:])
                    return

                nc.vector.tensor_reduce(
                    out=accs[:, 0:1], in_=zaccs[:], axis=mybir.AxisListType.X,
                    op=AL.add,
                )

                if stage == "gather":
                    nc.vector.memset(res[:], 0.0)
                    nc.sync.dma_start(out_d[:], res[:])
                    return

                # ---- finals
                nc.vector.tensor_tensor(
                    out=zdiv[:], in0=accs[:, 0:1], in1=ws8[:], op=AL.mult,
                )
                zz_ps = pred.tile([1, 1], f32, tag="y")
                nc.tensor.matmul(
                    zz_ps[:], lhsT=zdiv[:], rhs=ones128[0:BPC, :],
                    start=True, stop=True,
                )
                nc.vector.tensor_copy(res[:], zz_ps[:])
                nc.vector.tensor_scalar_mul(res[:], res[:], -1.0 / B)
                nc.sync.dma_start(out_d[:], res[:])

            if repeat == 1:
                body()
            else:
                with tc.For_i(0, repeat, 1):
                    body()


    _split_multi_waits(nc, mybir)
    # Populate .instr bytes for extended-inst InstISA subclasses (ap_gather);
    # without this the NEFF compiler sees empty .instr -> "ISA wrong length".
    mybir.codegen_inst_isa_subclasses(nc)
    return nc


def _shard_inputs(P, d_error, edge_weights, d_hw, edge_pairs):
    ep32 = edge_pairs.astype(np.int64, copy=False).view(np.int32).reshape(B, E, 4)
    derr = np.ascontiguousarray(d_error, dtype=np.float32)
    dhw = np.ascontiguousarray(d_hw, dtype=np.int32)
    in_maps = []
    for core in range(N_CORES):
        s = slice(BPC * core, BPC * (core + 1))
        in_maps.append({
            "p": np.ascontiguousarray(P[s], dtype=np.float32),
            "ep": np.ascontiguousarray(ep32[s]),
            "w": np.ascontiguousarray(edge_weights[s], dtype=np.float32),
            "derr": derr,
            "dhw": dhw,
        })
    return in_maps


def kernel(P, d_error, edge_weights, d_hw, edge_pairs):
    from concourse.bass_utils import run_bass_kernel_spmd

    nc = build_nc()
    in_maps = _shard_inputs(P, d_error, edge_weights, d_hw, edge_pairs)
    res = run_bass_kernel_spmd(nc, in_maps, core_ids=list(range(N_CORES)))
    total = np.float32(0.0)
    for r in res.results:
        total += np.float32(r["out"][0, 0])
    return np.float32(total)



# revision 3
# speedup vs baseline: 3.4400x; 3.4400x over previous
"""Trainium2 Bass kernel for AdjacencyErrorAwareLoss (hybrid).

Math (reference):
    A_fid = (d_hw == 1) * max(1 - d_error, 0)                    [128,128]
    scores[b,e] = P[b,i_e,:] @ A_fid @ P[b,j_e,:]                [B,E]
    loss = -mean_b( sum_e(w*scores) / max(sum_e w, 1e-8) )

With S_b = P_b @ A @ P_b^T:  sum_e w_e S_b[i_e,j_e] = <W_b, S_b>.

Hybrid edge split per sample (4096 edges):
  - first 3072 edges: matmul-scatter. W_b = sum_e w_e oh(i_e) oh(j_e)^T
    built on the PE from DVE-built one-hot chunk matrices
    (one fused tensor_scalar per chunk: (ramp == idx_p) * w_p, bf16),
    then <W_b, S_b> via one stt+accum per sample.
  - last 1024 edges: GPSIMD ap_gather from the masked flat-S table
    (the v0 design: partition 16c+q holds S_c rows [8q,8q+8) in a
    zeroed 16384-entry table; the 16-partition group sums to the true
    S value via a block-ones PE matmul), weighted sum via stt.
  Both per-sample partial sums are added before the wsum division.

Host-side sharding delivers indices/weights in DMA-friendly layouts
(contiguous per partition); the gather list comes pre-packed
(idx = i*128+j, int16) and pre-wrapped across each Q7 core's 16
partitions.

Distribution: data-parallel over B=64: 8 NeuronCores x 8 samples.
"""

import numpy as np

B, NL, NP, E = 64, 128, 128, 4096
N_CORES = 8
BPC = B // N_CORES   # samples per NeuronCore
EG = 1024            # gather-path edges per sample
EOH = E - EG         # one-hot-path edges per sample
KOH = EOH // 128     # one-hot chunks per sample (24)
GCH = 1              # gather calls (one 1024-idx list)


def _patch_tile_drain():
    """This toolchain's walrus rejects >1 sem wait on a Drain; split the
    kernel-tail drain into one drain per pending semaphore."""
    import concourse.tile as tile
    from concourse.vector_clock import ScopedClock, VectorClock

    def _drain_and_barrier_split(self, tick_clock, wait_clock):
        nc = self.nc
        gc = tick_clock.global_clock  # VectorClock
        n = len(gc)
        for p in [i for i in range(n) if gc[i] > 0]:
            vec = VectorClock([gc[i] if i == p else 0 for i in range(n)])
            drain_inst = nc.sync.drain()
            wait_clock.add_sem_waits(drain_inst.ins, ScopedClock({None: vec}))
        nc.all_engine_barrier()
        assert self.sems is not None
        popped = nc._tile_sem_poison_stack.pop()
        assert popped is self._sem_poison
        nc.clear_and_free_semaphores(list(self.sems.allocated().values()))
        nc.all_engine_barrier()

    tile.TileContext._drain_and_barrier = _drain_and_barrier_split


def _split_multi_waits(nc, mybir):
    """Walrus codegen accepts at most one sem wait per instruction ("Too
    many sync wait commands"). Hoist extra waits onto preceding same-engine
    NoOps (engines execute in order, so this blocks equivalently)."""
    k = 0
    for f in nc.m.functions:
        for bb in f.blocks:
            insts = list(bb.instructions)
            out = []
            changed = False
            for ins in insts:
                si = ins.sync_info
                waits = list(si.on_wait) if si is not None and si.on_wait else []
                if len(waits) > 1:
                    changed = True
                    for w in waits[:-1]:
                        nop = mybir.InstNoOp(name=f"xw-{k}", ins=[], outs=[])
                        k += 1
                        nop.engine = ins.engine
                        nop.sync_info = mybir.SyncInfo(on_wait=[w], on_update=[])
                        nc.register_instruction(nop)
                        out.append(nop)
                    ins.sync_info = mybir.SyncInfo(
                        on_wait=[waits[-1]], on_update=list(si.on_update or [])
                    )
                out.append(ins)
            if changed:
                bb.instructions = out


def build_nc(repeat: int = 1, stage: str = "full", var: str = "full"):
    import concourse.bass as bass
    import concourse.mybir as mybir
    import concourse.tile as tile
    from concourse import library_config

    _patch_tile_drain()

    AL = mybir.AluOpType
    f32 = mybir.dt.float32
    bf16 = mybir.dt.bfloat16
    i32 = mybir.dt.int32
    i16 = mybir.dt.int16

    nc = bass.Bass(detect_race_conditions=False)

    p_d = nc.dram_tensor("p", [BPC, NL, NP], f32, kind="ExternalInput")
    ii_d = nc.dram_tensor("idxi", [128, BPC * KOH], f32, kind="ExternalInput")
    jj_d = nc.dram_tensor("idxj", [128, BPC * KOH], f32, kind="ExternalInput")
    w_d = nc.dram_tensor("w", [128, BPC * 32], f32, kind="ExternalInput")
    gx_d = nc.dram_tensor("gidx", [128, EG // 16], i16, kind="ExternalInput")
    gw_d = nc.dram_tensor("gw", [BPC, EG], f32, kind="ExternalInput")
    derr_d = nc.dram_tensor("derr", [NP, NP], f32, kind="ExternalInput")
    dhw_d = nc.dram_tensor("dhw", [NP, NP], i32, kind="ExternalInput")
    out_d = nc.dram_tensor("out", [1, 1], f32, kind="ExternalOutput")

    ramp_np = np.tile(np.arange(128, dtype=np.float32)[None, :], (128, 1))
    ramp_d = nc.inline_tensor(ramp_np, name="ramp128")
    ones_d = nc.inline_tensor(np.ones((128, 1), dtype=np.float32), name="ones128")
    ident_d = nc.inline_tensor(np.eye(128, dtype=np.float32), name="ident128")
    blockones_np = np.zeros((128, BPC), dtype=np.float32)
    for c in range(BPC):
        blockones_np[16 * c:16 * (c + 1), c] = 1.0
    blockones_d = nc.inline_tensor(blockones_np, name="blockones")

    with tile.TileContext(nc) as tc:
        with (
            tc.tile_pool(name="persist", bufs=1) as persist,
            tc.tile_pool(name="oh", bufs=2) as ohp,
            tc.tile_pool(name="it", bufs=2) as itp,
            tc.tile_pool(name="it1", bufs=1) as itp1,
            tc.tile_pool(name="pp", bufs=2, space="PSUM") as pp,
            tc.tile_pool(name="wp", bufs=2, space="PSUM") as wpl,
            tc.tile_pool(name="fin", bufs=1, space="PSUM") as finp,
            tc.tile_pool(name="pred", bufs=2, space="PSUM") as pred,
            tc.tile_pool(name="sdram", bufs=2, space="DRAM") as sdram,
        ):
            nc.gpsimd.load_library(library_config.ap_gather)

            # ---- persistent tiles
            ramp_f = persist.tile([128, 128], f32)
            ramp = persist.tile([128, 128], bf16)
            ones128 = persist.tile([128, 1], f32)
            ident = persist.tile([128, 128], f32)
            blockones = persist.tile([128, BPC], f32)
            nc.sync.dma_start(ramp_f[:], ramp_d[:])
            nc.sync.dma_start(ones128[:], ones_d[:])
            nc.sync.dma_start(ident[:], ident_d[:])
            nc.sync.dma_start(blockones[:], blockones_d[:])
            nc.vector.tensor_copy(ramp[:], ramp_f[:])

            derr = persist.tile([128, 128], f32)
            dhw = persist.tile([128, 128], i32)
            nc.sync.dma_start(derr[:], derr_d[:])
            nc.sync.dma_start(dhw[:], dhw_d[:])

            table2 = persist.tile([128, 16384], f32)
            nc.vector.memset(table2[:, 0:8192], 0.0)
            nc.scalar.memzero(table2[:, 8192:16384])

            def body(_it=0):
                pall = itp1.tile([128, BPC, 128], f32, tag="pall")
                ptbf = itp1.tile([128, BPC, 128], bf16, tag="ptbf")
                vbf = itp1.tile([128, BPC, 128], bf16, tag="vbf")
                s_all = itp1.tile([128, BPC, 128], f32, tag="s_all")
                sbf = itp.tile([128, BPC, 128], bf16, tag="sbf")
                wsb = itp1.tile([128, BPC, 128], bf16, tag="wsb")
                idxi = itp.tile([128, BPC * KOH], f32, tag="idxi")
                idxj = itp.tile([128, BPC * KOH], f32, tag="idxj")
                wpp = itp.tile([128, BPC * 32], f32, tag="wpp")
                idx16 = itp.tile([128, EG // 16], i16, tag="idx16")
                gw = itp.tile([BPC, EG], f32, tag="gw")
                val2s = [itp1.tile([128, EG], f32, tag=f"val2_{i}",
                                   name=f"val2_{i}")
                         for i in range(GCH)]
                zg = itp.tile([BPC, EG // 512], f32, tag="zg")
                scg = itp.tile([BPC, EG], f32, tag="scg")
                afid_b = itp1.tile([128, 128], bf16, tag="afid_b")
                relu_e = itp1.tile([128, 128], f32, tag="relu_e")
                zw = itp.tile([128, 2 * BPC], f32, tag="zw")
                scr = itp.tile([128, 128], bf16, tag="scr")
                z8 = itp.tile([BPC, 1], f32, tag="z8")
                ws8 = itp.tile([BPC, 1], f32, tag="ws8")
                zdiv = itp.tile([BPC, 1], f32, tag="zdiv")
                res = itp.tile([1, 1], f32, tag="res")
                # ---- contiguous input DMAs (gather idx first: gathers are
                # the longest fixed-rate chain after the table is up)
                nc.sync.dma_start(idx16[:], gx_d[:])
                nc.sync.dma_start(idxi[:], ii_d[:])
                nc.sync.dma_start(idxj[:], jj_d[:])
                nc.scalar.dma_start(wpp[:], w_d[:])
                nc.scalar.dma_start(gw[:], gw_d[:])
                p_src = bass.AP(
                    tensor=p_d, offset=0,
                    ap=[[128, 128], [NL * NP, BPC], [1, 128]],
                )
                nc.sync.dma_start(pall[:], p_src)

                # ---- A_fid = (dhw == 1) * relu(1 - derr); bf16 copy
                nc.scalar.activation(
                    relu_e[:], derr[:],
                    mybir.ActivationFunctionType.Relu, bias=1.0, scale=-1.0,
                )
                nc.vector.scalar_tensor_tensor(
                    out=afid_b[:], in0=dhw[:], scalar=1, in1=relu_e[:],
                    op0=AL.is_equal, op1=AL.mult,
                )

                # ---- wsum partials
                nc.vector.tensor_reduce(
                    out=zw[:, BPC:2 * BPC],
                    in_=wpp[:].rearrange("p (c k) -> p c k", k=32),
                    axis=mybir.AxisListType.X, op=AL.add,
                )

                # ---- P^T; V = A^T P^T; S = V^T P^T (PE, bf16)
                for c in range(BPC):
                    pt_ps = pp.tile([128, 128], f32, tag="ps")
                    nc.tensor.transpose(pt_ps[:], pall[:, c, :], ident[:])
                    nc.scalar.copy(ptbf[:, c, :], pt_ps[:])
                for h in range(2):
                    v_ps = pp.tile([128, 512], f32, tag="ps")
                    nc.tensor.matmul(
                        v_ps[:],
                        lhsT=afid_b[:],
                        rhs=ptbf[:].rearrange("p a b -> p (a b)")[:, 512 * h:512 * (h + 1)],
                        start=True, stop=True,
                    )
                    nc.scalar.copy(
                        vbf[:].rearrange("p a b -> p (a b)")[:, 512 * h:512 * (h + 1)],
                        v_ps[:],
                    )
                # S: f32 copy for the gather table + bf16 copy for the stt;
                # bounce S to DRAM per-sample so stripe reads can start early
                s_dr = sdram.tile([128, BPC, 128], f32, tag="sdram")
                for c in range(BPC):
                    s_ps = pp.tile([128, 128], f32, tag="ps")
                    nc.tensor.matmul(
                        s_ps[:], lhsT=vbf[:, c, :], rhs=ptbf[:, c, :],
                        start=True, stop=True,
                    )
                    nc.scalar.copy(s_all[:, c, :], s_ps[:])
                    nc.scalar.copy(sbf[:, c, :], s_all[:, c, :])
                    if stage != "ohonly":
                        eng = nc.sync if c % 2 == 0 else nc.scalar
                        eng.dma_start(s_dr[:, c, :], s_all[:, c, :])
                if stage != "ohonly":
                    # 16 stripe reads: table2[{16c+q}, 1024q:1024q+1024] =
                    #   S_c rows [8q, 8q+8)
                    for q in range(16):
                        dst = table2[q::16, 1024 * q:1024 * (q + 1)]
                        eng = (nc.sync, nc.scalar)[q % 2]
                        eng.dma_start(
                            dst,
                            s_dr[:].rearrange("l c x -> c l x")[:, 8 * q:8 * q + 8, :],
                        )

                if stage == "loads":
                    nc.vector.memset(res[:], 0.0)
                    nc.sync.dma_start(out_d[:], res[:])
                    return

                # ---- gather chunks (GPSIMD only; reductions issued after
                # the one-hot loop so PE/DVE don't stall waiting on them)
                for g in range(GCH if stage != "ohonly" else 0):
                    nc.gpsimd.ap_gather(
                        out_ap=val2s[g][:].unsqueeze(2),
                        in_ap=table2[:].unsqueeze(2),
                        idxs_ap=idx16[:],
                        channels=128, num_elems=16384, d=1,
                        num_idxs=EG,
                    )

                # ---- one-hot path: per sample chunks + W matmuls + <W,S>
                for c in range(BPC if stage != "gonly" else 0):
                    uw = ohp.tile([128, KOH, 128], bf16, tag="uw")
                    v01 = ohp.tile([128, KOH, 128], bf16, tag="v01")
                    for k in range(KOH):
                        ck = c * KOH + k
                        if var == "constsc":
                            s1, s2, s3 = 7.0, 0.5, 9.0
                        else:
                            s1 = idxi[:, ck:ck + 1]
                            s2 = wpp[:, c * 32 + k:c * 32 + k + 1]
                            s3 = idxj[:, ck:ck + 1]
                        nc.vector.tensor_scalar(
                            out=uw[:, k, :], in0=ramp[:],
                            scalar1=s1, scalar2=s2,
                            op0=AL.is_equal, op1=AL.mult,
                        )
                        nc.vector.tensor_scalar(
                            out=v01[:, k, :], in0=ramp[:],
                            scalar1=s3, scalar2=None,
                            op0=AL.is_equal,
                        )
                    w_ps = wpl.tile([128, 128], f32, tag="wc")
                    for k in range(KOH):
                        nc.tensor.matmul(
                            w_ps[:], lhsT=uw[:, k, :], rhs=v01[:, k, :],
                            start=(k == 0), stop=(k == KOH - 1),
                        )
                    nc.scalar.copy(wsb[:, c, :], w_ps[:])
                    nc.vector.scalar_tensor_tensor(
                        out=scr[:], in0=wsb[:, c, :], scalar=0.0,
                        in1=sbf[:, c, :], op0=AL.add, op1=AL.mult,
                        accum_out=zw[:, c:c + 1],
                    )

                # ---- gather reductions (PE block-ones sum + weighted stt)
                if stage != "ohonly":
                    for h in range(EG // 512):
                        y_ps = pred.tile([BPC, 512], f32, tag="y")
                        nc.tensor.matmul(
                            y_ps[:], lhsT=blockones[:],
                            rhs=val2s[0][:, 512 * h:512 * (h + 1)],
                            start=True, stop=True,
                        )
                        nc.vector.scalar_tensor_tensor(
                            out=scg[:, 0:512], in0=y_ps[:], scalar=0.0,
                            in1=gw[:, 512 * h:512 * (h + 1)],
                            op0=AL.add, op1=AL.mult,
                            accum_out=zg[:, h:h + 1],
                        )

                # ---- partition reductions via PE; combine with gather z
                red_ps = finp.tile([BPC, 1], f32, tag="red")
                nc.tensor.matmul(
                    red_ps[:], lhsT=zw[:, 0:BPC], rhs=ones128[:],
                    start=True, stop=True,
                )
                nc.scalar.copy(z8[:], red_ps[:])
                red_ps2 = finp.tile([BPC, 1], f32, tag="red")
                nc.tensor.matmul(
                    red_ps2[:], lhsT=zw[:, BPC:2 * BPC], rhs=ones128[:],
                    start=True, stop=True,
                )
                nc.scalar.copy(ws8[:], red_ps2[:])

                # z_c += gather partials; wsum_c += gather w partials
                if stage == "ohonly":
                    nc.vector.memset(zg[:], 0.0)
                if stage == "gonly":
                    nc.vector.memset(zw[:, 0:BPC], 0.0)
                nc.vector.tensor_reduce(
                    out=zg[:, 0:1], in_=zg[:], axis=mybir.AxisListType.X,
                    op=AL.add,
                )
                nc.vector.tensor_tensor(
                    out=z8[:], in0=z8[:], in1=zg[:, 0:1], op=AL.add,
                )

                # ---- finals
                nc.vector.tensor_scalar(
                    out=ws8[:], in0=ws8[:], scalar1=1e-8, scalar2=None, op0=AL.max,
                )
                nc.vector.reciprocal(ws8[:], ws8[:])
                nc.vector.tensor_tensor(
                    out=zdiv[:], in0=z8[:], in1=ws8[:], op=AL.mult,
                )
                zz_ps = finp.tile([1, 1], f32, tag="zz")
                nc.tensor.matmul(
                    zz_ps[:], lhsT=zdiv[:], rhs=ones128[0:BPC, :],
                    start=True, stop=True,
                )
                nc.scalar.mul(res[:], zz_ps[:], -1.0 / B)
                nc.sync.dma_start(out_d[:], res[:])

            if repeat == 1:
                body()
            elif repeat == 2:
                body(0)
                body(1)
            else:
                with tc.For_i(0, repeat // 2, 1):
                    body(0)
                    body(1)
                for _ex in range(repeat % 2):
                    body(0)

    _split_multi_waits(nc, mybir)
    mybir.codegen_inst_isa_subclasses(nc)
    return nc


def _shard_inputs(P, d_error, edge_weights, d_hw, edge_pairs):
    iif = edge_pairs[..., 0].astype(np.float32)   # [B, E]
    jjf = edge_pairs[..., 1].astype(np.float32)
    wwf = edge_weights.astype(np.float32)
    # one-hot path: first EOH edges -> [128, BPC*KOH], x[p, c*KOH+k] =
    # orig[c, 128k+p]
    ii = iif[:, :EOH].reshape(B, KOH, 128)
    jj = jjf[:, :EOH].reshape(B, KOH, 128)
    ww = np.concatenate([
        wwf[:, :EOH].reshape(B, KOH, 128),
        wwf[:, EOH:].reshape(B, 8, 128),
    ], axis=1)  # [B, 32, 128]
    # gather path: last EG edges; list position k = local edge index g;
    # wrapped: idx16[16c + k%16, k//16] = i*128 + j  (int16)
    gi = edge_pairs[:, EOH:, 0].astype(np.int64)
    gj = edge_pairs[:, EOH:, 1].astype(np.int64)
    gidx_full = (gi * 128 + gj).astype(np.int16)  # [B, EG]
    gwf = wwf[:, EOH:]                            # [B, EG]

    derr = np.ascontiguousarray(d_error, dtype=np.float32)
    dhw = np.ascontiguousarray(d_hw, dtype=np.int32)
    in_maps = []
    for core in range(N_CORES):
        s = slice(BPC * core, BPC * (core + 1))
        g16 = np.zeros((128, EG // 16), dtype=np.int16)
        for c in range(BPC):
            row = gidx_full[BPC * core + c]       # [EG]
            g16[16 * c + (np.arange(EG) % 16), np.arange(EG) // 16] = row
        in_maps.append({
            "p": np.ascontiguousarray(P[s], dtype=np.float32),
            "idxi": np.ascontiguousarray(
                ii[s].transpose(2, 0, 1).reshape(128, BPC * KOH)),
            "idxj": np.ascontiguousarray(
                jj[s].transpose(2, 0, 1).reshape(128, BPC * KOH)),
            "w": np.ascontiguousarray(
                ww[s].transpose(2, 0, 1).reshape(128, BPC * 32)),
            "gidx": g16,
            "gw": np.ascontiguousarray(gwf[s]),
            "derr": derr,
            "dhw": dhw,
        })
    return in_maps


def kernel(P, d_error, edge_weights, d_hw, edge_pairs):
    from concourse.bass_utils import run_bass_kernel_spmd

    nc = build_nc()
    in_maps = _shard_inputs(P, d_error, edge_weights, d_hw, edge_pairs)
    res = run_bass_kernel_spmd(nc, in_maps, core_ids=list(range(N_CORES)))
    total = np.float32(0.0)
    for r in res.results:
        total += np.float32(r["out"][0, 0])
    return np.float32(total)
